# revision 20
# baseline (speedup 1.0000x reference)
"""Bass/Tile Trainium2 kernel for nn_BcosGCNLayer (b-cos linear layer, B=2).

reference:
    lin  = z @ W.T
    cos  = normalize(z) @ normalize(W).T
    out  = lin * |cos|**(B-1) = lin * |cos|          (B = 2)

Identity: with W~ = W * ||w_row||^(-1/2) and z~ = z * ||z_row||^(-1/2),
    P = z~ @ W~.T  ==>  P * |P| = lin * |cos| = out.
Both norm scalings are folded into the operands on the host, so the device
program is exactly one GEMM plus a two-op epilogue; there are no
transposes, reductions, or scale operands on device.

The GEMM is **weight-stationary**: the stationary operand is a [128i,128o]
block of W~T, streamed against four 512-row z~ chunks before switching.
The tile legalizer emits one InstLdweights per matmul; a post-pass
(_dedupe_ldweights) removes reloads of the identical weight block, which
matters because each LDWEIGHTS serializes with the matmul stream (~128
cycles: measured 270 ns/MM self-loading vs 213 ns pure stream). Output
comes out transposed [o, n]; the host transposes it back (the |P|*P
epilogue is elementwise, so orientation is free).

Layouts (host-prepared):
  zt [128, 4, rows]  : fp8 e3m4, zt[p, k, n] = ALPHA * z~[n, 128*k + p]
                       (z~ transposed into the matmul moving-operand
                       layout). ALPHA=8 keeps the quantized values in
                       e3m4's normal range (min normal 0.25; z~ alone has
                       std ~0.21 and would quantize mostly subnormal:
                       measured 1.8e-2 vs 1.15e-2 global rel err). The
                       e3m4 x bf16 mixed-dtype matmul runs at full PE rate
                       and was verified bit-exact on hardware; vs bf16 z
                       it raises rel err 3.0e-3 -> 1.15e-2 (gate 2e-2)
                       and halves the z DMA bytes.
  wt [128, 4, 512]   : bf16, wt[p, k, o] = W~[o, 128*k + p] / ALPHA.
  out [512, rows]    : transposed, bf16; host upcasts + transposes back.

Per 2048-row supergroup: for each of 4 o-blocks, each of 4 k-blocks is
loaded once into the PE and streamed against the supergroup's four
512-row z chunks, accumulating into 4 PSUM banks ([128,2,512] pairs);
ACT computes |P| on a [128,1024] pair (PSUM->SBUF; a DVE op may read only
one PSUM operand) and DVE multiplies P*|P| into the bf16 store buffer.
Loads ride the sync (SP) HWDGE queue in up-to-64-tile chunks
(8KB/partition descriptors); stores ride the gpsimd SWDGE queue per
(supergroup, o-block) -- [128, 2048] bf16, 4KB/partition descriptors.

Sharding: data-parallel rows across 8 cores (12500 rows/core padded to
12544 = 98*128); weight replicated.
"""

import numpy as np
import ml_dtypes

import concourse.bacc as bacc
import concourse.bass as bass
import concourse.mybir as mybir
import concourse.tile as tile

P = 128
D = 512
KB = D // P  # 4 contraction blocks of 128
NC_T = 4  # tiles per n-chunk (512 rows = max moving free dim)
SG = 16  # tiles per supergroup (4 n-chunks)
MAX_CHUNK = 64  # tiles per steady-state z-load chunk (8KB/partition fp8 runs)
CHUNK_SCHEDULE = (4, 8, 16, 32)  # ramp-in chunk sizes, then MAX_CHUNK
N_CORES = 8
TOTAL_ROWS = 100000
ROWS_PER_CORE_RAW = TOTAL_ROWS // N_CORES  # 12500
TILES_PER_CORE = -(-ROWS_PER_CORE_RAW // P)  # 98
ROWS_PER_CORE = TILES_PER_CORE * P  # 12544

F32 = mybir.dt.float32
BF16 = mybir.dt.bfloat16
Z_DT = mybir.dt.float8e3  # z ships as fp8 e3m4 (4 mantissa bits)
AL = mybir.AluOpType
ACT = mybir.ActivationFunctionType
BF16_NP = ml_dtypes.bfloat16
Z_NP = ml_dtypes.float8_e3m4
ALPHA = 8.0  # z~ prescale (folded out of W~): keeps e3m4 z in normal range


def _chunks(n_tiles):
    """Load-chunk schedule: small chunks first (fast PE ramp-in), then
    MAX_CHUNK. All sizes/starts are multiples of NC_T so n-chunks never
    straddle load chunks."""
    cs, t, i = [], 0, 0
    while t < n_tiles:
        want = CHUNK_SCHEDULE[i] if i < len(CHUNK_SCHEDULE) else MAX_CHUNK
        ct = min(want, n_tiles - t)
        cs.append((t, ct))
        t += ct
        i += 1
    return cs


def _supergroups(n_tiles):
    """[(tile0, [n-chunk widths in tiles])] covering all tiles."""
    sgs, t = [], 0
    while t < n_tiles:
        st = min(SG, n_tiles - t)
        ncs = []
        u = 0
        while u < st:
            ncs.append(min(NC_T, st - u))
            u += NC_T
        sgs.append((t, ncs))
        t += st
    return sgs


def _dedupe_ldweights(nc) -> int:
    """Drop InstLdweights that reload the stationary operand already in
    the PE array (identical SBUF access pattern as the previous load in
    the same basic block). Any waits/updates they carry are merged into
    the next InstMatmult; generate_event_semaphores (later, inside
    nc.compile()) legalizes multi-wait instructions."""
    removed = 0
    for blk in nc.m.functions[0].blocks:
        out = []
        last_sig = None
        pend_w, pend_u = [], []
        for ins in blk.instructions:
            t = type(ins).__name__
            if t == "InstLdweights":
                ap = ins.ins[0]
                sig = (
                    str(ap.memref), ap.offset, str(ap.ap), str(ap.dtype),
                    str(ins.is_transpose), str(ins.perf_mode),
                    str(ins.tile_position),
                )
                if sig == last_sig:
                    si = ins.sync_info
                    if si is not None:
                        pend_w.extend(si.on_wait or [])
                        pend_u.extend(si.on_update or [])
                    removed += 1
                    continue
                last_sig = sig
            elif t == "InstMatmult":
                if pend_w or pend_u:
                    si = ins.sync_info
                    for w in pend_w:
                        si.on_wait.append(w)
                    for u in pend_u:
                        si.on_update.append(u)
                    pend_w, pend_u = [], []
            out.append(ins)
        assert not pend_w and not pend_u, "dropped ldweights with no successor matmul"
        blk.instructions = out
    return removed


def build_kernel(
    rows: int = ROWS_PER_CORE,
    repeat: int = 1,
    alias_rows: int = 0,
    hw_loop: int = 0,
) -> bass.Bass:
    """Per-core program: zt [128,4,rows], wt [128,4,512] -> out [512,rows].

    repeat / alias_rows / hw_loop are bench-only knobs: alias_rows shrinks
    the DRAM tensors (addressing wraps) so host<->device shipping is tiny,
    hw_loop wraps the whole pass in a For_i, repeat emits several passes
    per loop iteration.
    """
    assert rows % P == 0
    n_tiles = rows // P
    dram_rows = alias_rows or rows
    assert dram_rows % P == 0

    chunks = _chunks(n_tiles)
    sgs = _supergroups(n_tiles)

    nc = bacc.Bacc()
    zt_dram = nc.dram_tensor("zt", [P, KB, dram_rows], Z_DT, kind="ExternalInput")
    wt_dram = nc.dram_tensor("wt", [P, KB, D], BF16, kind="ExternalInput")
    out_dram = nc.dram_tensor("out", [D, dram_rows], BF16, kind="ExternalOutput")

    with tile.TileContext(nc) as tc:
        with (
            tc.tile_pool(name="wtp", bufs=1) as wt_pool,
            tc.tile_pool(name="zin", bufs=3) as zin_pool,
            tc.tile_pool(name="outb", bufs=4) as out_pool,
            tc.tile_pool(name="absb", bufs=4) as ab_pool,
            tc.tile_pool(name="psum", bufs=4, space=bass.MemorySpace.PSUM) as pt_pool,
        ):
            wT = wt_pool.tile([P, KB, D], BF16)
            nc.scalar.dma_start(wT[:], wt_dram[:])

            # Preload the Abs activation table (~1.3us) while DMAs stream.
            pre = wt_pool.tile([P, 1], F32)
            nc.vector.memset(pre[:], 0.0)
            nc.scalar.activation(pre[:], pre[:], ACT.Abs)

            # Dependency-free PE warmup: burn the p-state ramp on junk
            # matmuls while the first z chunk is in flight (fp32 rate is
            # 4 cyc/row -- plenty of ramp cycles from 3 instructions).
            junk = wt_pool.tile([P, D], F32)
            nc.vector.memset(junk[:], 0.0)
            warm = pt_pool.tile([P, 2, D], F32, name="pt")
            for _ in range(3):
                nc.tensor.matmul(warm[:, 0, :], junk[:, :P], junk[:])

            tile_chunk = {}  # tile index -> chunk index
            for ci, (c0, ct) in enumerate(chunks):
                for ti in range(ct):
                    tile_chunk[c0 + ti] = ci

            def emit_pass():
                zc_tiles = {}

                def load_chunk(ci):
                    c0, ct = chunks[ci]
                    zc = zin_pool.tile([P, KB, MAX_CHUNK * P], Z_DT, name="zc")
                    s0 = (c0 * P) % dram_rows
                    if s0 + ct * P > dram_rows:  # alias-mode wrap clamp
                        s0 = 0
                    nc.sync.dma_start(
                        zc[:, :, : ct * P], zt_dram[:, :, s0 : s0 + ct * P]
                    )
                    zc_tiles[ci] = (zc, c0)

                # Emit every chunk load upfront: the SP queue is FIFO and
                # each DMA's buffer-free semaphore throttles it, so this is
                # maximal prefetch depth for free.
                for ci in range(len(chunks)):
                    load_chunk(ci)

                for t0, ncs in sgs:
                    n_pairs = (len(ncs) + 1) // 2
                    sg_cols = sum(ncs) * P
                    for o in range(KB):
                        pss = [
                            pt_pool.tile([P, 2, D], F32, name="pt")
                            for _ in range(n_pairs)
                        ]
                        for k in range(KB):
                            for j, nct in enumerate(ncs):
                                t = t0 + j * NC_T
                                zc, c0 = zc_tiles[tile_chunk[t]]
                                off = (t - c0) * P
                                w = nct * P
                                nc.tensor.matmul(
                                    pss[j // 2][:, j % 2, :w],
                                    wT[:, k, o * P : (o + 1) * P],
                                    zc[:, k, off : off + w],
                                    start=(k == 0),
                                    stop=(k == KB - 1),
                                )
                        # |P| on ACT (PSUM->SBUF), P*|P| on DVE: a DVE op
                        # may read at most one non-scalar PSUM operand.
                        # Two n-chunks (2 PSUM banks) per op.
                        og = out_pool.tile([P, (SG // NC_T) * D], BF16, name="og")
                        pos = 0
                        for j2 in range(n_pairs):
                            w2 = ncs[2 * j2] * P
                            if 2 * j2 + 1 < len(ncs):
                                w2 += ncs[2 * j2 + 1] * P
                            if w2 == 2 * D:
                                src = pss[j2][:].rearrange("p a b -> p (a b)")
                            else:
                                src = pss[j2][:, 0, :w2]
                            ab = ab_pool.tile([P, 2 * D], F32, name="ab")
                            nc.scalar.activation(ab[:, :w2], src, ACT.Abs)
                            nc.vector.tensor_mul(
                                og[:, pos : pos + w2], src, ab[:, :w2]
                            )
                            pos += w2
                        r0 = (t0 * P) % dram_rows
                        if r0 + sg_cols > dram_rows:  # alias-mode wrap clamp
                            r0 = 0
                        nc.gpsimd.dma_start(
                            out_dram[o * P : (o + 1) * P, r0 : r0 + sg_cols],
                            og[:, :sg_cols],
                        )

            if hw_loop:
                with tc.For_i(
                    0,
                    hw_loop,
                    1,
                    hint_engines=(
                        mybir.EngineType.PE,
                        mybir.EngineType.Activation,
                        mybir.EngineType.DVE,
                        mybir.EngineType.SP,
                        mybir.EngineType.Pool,
                    ),
                ):
                    for _ in range(repeat):
                        emit_pass()
            else:
                for _ in range(repeat):
                    emit_pass()

    _dedupe_ldweights(nc)
    nc.compile()
    return nc


_NC_CACHE: dict = {}


def _get_nc(rows: int) -> bass.Bass:
    if rows not in _NC_CACHE:
        _NC_CACHE[rows] = build_kernel(rows)
    return _NC_CACHE[rows]


def prep_in_maps(z: np.ndarray, weight: np.ndarray):
    """Host prep: fold norms into operands, transpose into device
    layouts, cast (z: fp8 e3m4, w: bf16), shard rows across cores."""
    z = np.ascontiguousarray(z, dtype=np.float32)
    weight = np.ascontiguousarray(weight, dtype=np.float32)
    n_rows = z.shape[0]
    per_core = -(-n_rows // N_CORES)
    per_core_pad = -(-per_core // P) * P

    # W~ = W * ||w_row||^(-1/2) / ALPHA, transposed into [p, k, o]
    wn = np.sqrt((weight.astype(np.float64) ** 2).sum(axis=1))
    wt_f = (weight * (wn**-0.5)[:, None].astype(np.float32)).T / ALPHA
    wt_host = np.ascontiguousarray(
        wt_f.reshape(KB, P, D).transpose(1, 0, 2).astype(BF16_NP)
    )

    # z~ = z * ||z_row||^(-1/2) * ALPHA
    zn = np.sqrt((z * z).sum(axis=1, dtype=np.float64))
    zs = np.where(zn > 0, ALPHA * zn**-0.5, 1.0).astype(np.float32)

    in_maps = []
    for c in range(N_CORES):
        lo = c * per_core
        hi = min(n_rows, (c + 1) * per_core)
        shard = np.zeros((per_core_pad, D), dtype=np.float32)
        shard[: hi - lo] = z[lo:hi] * zs[lo:hi, None]
        zt_host = np.ascontiguousarray(
            shard.T.reshape(KB, P, per_core_pad).transpose(1, 0, 2).astype(Z_NP)
        )
        in_maps.append({"zt": zt_host, "wt": wt_host})
    return in_maps, per_core, per_core_pad


def kernel(z: np.ndarray, weight: np.ndarray) -> np.ndarray:
    """Full-input entry point: z [100000, 512] f32, weight [512, 512] f32."""
    from concourse.bass_utils import run_bass_kernel_spmd

    n_rows = z.shape[0]
    in_maps, per_core, per_core_pad = prep_in_maps(z, weight)
    nc = _get_nc(per_core_pad)

    res = run_bass_kernel_spmd(nc, in_maps, core_ids=list(range(N_CORES)))
    out = np.empty((n_rows, D), dtype=np.float32)
    for c in range(N_CORES):
        lo = c * per_core
        hi = min(n_rows, (c + 1) * per_core)
        out[lo:hi] = res.results[c]["out"][:, : hi - lo].astype(np.float32).T
    return out
